# revision 2
# baseline (speedup 1.0000x reference)
"""Trainium2 Bass kernel for a bidirectional cross-attention layer (v3).

Per batch sample (one NeuronCore each, 8 samples / 8 cores):
    e  = seq_1 @ seq_2^T                     [L, L]
    P  = exp(e)            (no max-subtraction: |e| <~ 70 << fp32 overflow)
    seq_1_hat = diag(1/rowsum(P)) @ P   @ seq_2
    seq_2_hat = diag(1/colsum(P)) @ P^T @ seq_1

v3: scores computed ONCE (bf16); P^T comes from the XBAR DMA-transpose
(2-byte dtypes, 16x128 tiles) instead of a second score pass, so the PE
runs only the three real GEMMs and zero transposes. colsum via DVE
free-dim reduces over the transposed P stripes. Output transposes also
via XBAR (bf16), fp32 normalization applied afterwards on DVE.
"""

import os

os.environ.setdefault("MYCRO_LOCAL_CACHE", "1")

import numpy as np

import concourse.mybir as mybir
from concourse import bacc
from concourse.bass_utils import run_bass_kernel_spmd
from concourse.tile import TileContext

B, L, D = 8, 2048, 128
NBLK = L // 128  # 16 blocks of 128
NCH = L // 512   # 4 chunks of 512

F32 = mybir.dt.float32
BF16 = mybir.dt.bfloat16
AF = mybir.ActivationFunctionType
ALU = mybir.AluOpType
AX = mybir.AxisListType


def _build():
    nc = bacc.Bacc(
        "TRN2", target_bir_lowering=False, debug=False, enable_asserts=False
    )
    s1 = nc.dram_tensor("seq_1", [L, D], F32, kind="ExternalInput").ap()
    s2 = nc.dram_tensor("seq_2", [L, D], F32, kind="ExternalInput").ap()
    o1 = nc.dram_tensor("out1", [L, D], F32, kind="ExternalOutput").ap()
    o2 = nc.dram_tensor("out2", [L, D], F32, kind="ExternalOutput").ap()

    with TileContext(nc) as tc:
        with (
            tc.tile_pool(name="big", bufs=1) as big,
            tc.tile_pool(name="pstr", bufs=3) as pstr,
            tc.tile_pool(name="acc2p", bufs=1, space="PSUM") as acc2p,
            tc.tile_pool(name="outp", bufs=4) as outp,
        ):
            # ---- persistent SBUF tensors -------------------------------
            s1f = big.tile([128, L], F32, tag="s1f")    # [i%128, (blk,d)]
            s2f = big.tile([128, L], F32, tag="s2f")
            s1b = big.tile([128, L], BF16, tag="s1b")   # bf16 casts
            s2b = big.tile([128, L], BF16, tag="s2b")
            s1t = big.tile([128, NBLK, 128], BF16, tag="s1t")  # [d, blk, i%128]
            s2t = big.tile([128, NBLK, 128], BF16, tag="s2t")
            ptp = big.tile([128, NBLK, L], BF16, tag="ptp")  # [j%128, cblk, i]
            rsum4 = big.tile([128, NBLK * 2], F32, tag="rsum4")
            csum4 = big.tile([128, NBLK * NBLK], F32, tag="csum4")
            rowsum = big.tile([128, NBLK], F32, tag="rowsum")
            colsum = big.tile([128, NBLK], F32, tag="colsum")
            rrow = big.tile([128, NBLK], F32, tag="rrow")
            rcol = big.tile([128, NBLK], F32, tag="rcol")

            # ---- preload -----------------------------------------------
            # s2 first (scores need the FULL s2t before block 0).
            # s2 plumbing on the SP queue, s1 on the Activation queue so
            # the triggers issue in parallel.
            for t_dram, t_f in ((s2, s2f), (s1, s1f)):
                for g in range(4):  # 4 row-blocks per DMA
                    sl = slice(g * 512, (g + 1) * 512)
                    nc.sync.dma_start(
                        t_f[:, sl].rearrange("p (blk d) -> p blk d", blk=4),
                        t_dram[sl, :].rearrange("(blk p) d -> p blk d", blk=4),
                    )
            for t_f, t_b, t_t in ((s2f, s2b, s2t), (s1f, s1b, s1t)):
                for g in range(4):
                    sl = slice(g * 512, (g + 1) * 512)
                    nc.vector.tensor_copy(t_b[:, sl], t_f[:, sl])
                # [i%128,(blk,d)] --bf16 xbar--> [d, blk, i%128]
                nc.sync.dma_start(t_t[:, 0:8, :], t_b[:, :1024], transpose=True)
                nc.sync.dma_start(t_t[:, 8:16, :], t_b[:, 1024:], transpose=True)

            # ---- phase A: per i-block ----------------------------------
            #   e row-stripe -> exp -> P_b (bf16) (+rowsum accum)
            #   o2T[d, j] += s1b[:,b]^T @ P_b   (PSUM, accumulated)
            #   XBAR-transpose P_b into ptp[j%128, c, i]
            #   DVE partial colsum over ptp i-slice b
            acc2 = acc2p.tile([128, L], F32, tag="acc2")
            with tc.tile_pool(name="ep", bufs=2, space="PSUM") as ep:
                for b in range(NBLK):
                    bsl = slice(b * 128, (b + 1) * 128)
                    pb = pstr.tile([128, L], BF16, tag="pb")
                    for h in range(2):
                        et = ep.tile([128, 1024], F32, tag="et")
                        for q in range(2):
                            a = 2 * h + q
                            nc.tensor.matmul(
                                et[:, q * 512:(q + 1) * 512],
                                lhsT=s1t[:, b, :],
                                rhs=s2t[:, 4 * a:4 * a + 4, :],
                                start=True, stop=True,
                            )
                        k = b * 2 + h
                        nc.scalar.activation(
                            pb[:, h * 1024:(h + 1) * 1024], et, AF.Exp,
                            accum_out=rsum4[:, k:k + 1],
                        )
                    for c in range(NCH):
                        csl = slice(c * 512, (c + 1) * 512)
                        nc.tensor.matmul(
                            acc2[:, csl],
                            lhsT=s1b[:, bsl],
                            rhs=pb[:, csl],
                            start=(b == 0), stop=(b == NBLK - 1),
                        )
                    nc.sync.dma_start(ptp[:, :, bsl], pb, transpose=True)
                    nc.vector.tensor_reduce(
                        csum4[:, b * NBLK:(b + 1) * NBLK],
                        ptp[:, :, bsl], axis=AX.X, op=ALU.add,
                    )

                # rowsum + reciprocal
                nc.vector.tensor_reduce(
                    rowsum, rsum4.rearrange("p (b t) -> p b t", b=NBLK),
                    axis=AX.X, op=ALU.add,
                )
                nc.vector.reciprocal(rrow, rowsum)
                # colsum: fold the 16 per-b partials (strided view) + recip
                nc.vector.tensor_reduce(
                    colsum, csum4.rearrange("p (b c) -> p c b", b=NBLK),
                    axis=AX.X, op=ALU.add,
                )
                nc.vector.reciprocal(rcol, colsum)

            # ---- phase B: o1T chunks + both epilogues ------------------
            # Chunk 3 first: its matmuls depend on the LAST XBAR stripe,
            # which naturally drains the et pipeline before acc1 reuses
            # those PSUM banks.
            with tc.tile_pool(name="acc1p", bufs=2, space="PSUM") as acc1p:
                def epilogue2(chunk, acc1):
                    """Transpose+normalize+store the o1 (acc1) and o2
                    (acc2 slice) chunks with a single shared XBAR."""
                    isl = slice(chunk * 512, (chunk + 1) * 512)
                    bb = outp.tile([128, 1024], BF16, tag="ebf")
                    nc.vector.tensor_copy(bb[:, :512], acc1)
                    nc.vector.tensor_copy(bb[:, 512:], acc2[:, isl])
                    tb = outp.tile([128, 8, 128], BF16, tag="etr")
                    nc.sync.dma_start(tb, bb, transpose=True)
                    of = outp.tile([128, 1024], F32, tag="eo")
                    for half, rinv in ((0, rrow), (1, rcol)):
                        for c2 in range(4):
                            blk = chunk * 4 + c2
                            nc.vector.tensor_scalar_mul(
                                of[:, half * 512 + c2 * 128:
                                   half * 512 + (c2 + 1) * 128],
                                tb[:, half * 4 + c2, :], rinv[:, blk:blk + 1],
                            )
                    for half, dram in ((0, o1), (1, o2)):
                        nc.sync.dma_start(
                            dram[chunk * 512:(chunk + 1) * 512, :]
                            .rearrange("(c p) d -> p c d", c=4),
                            of[:, half * 512:(half + 1) * 512]
                            .rearrange("p (c d) -> p c d", c=4),
                        )

                for chunk in (3, 0, 1, 2):
                    acc1 = acc1p.tile([128, 512], F32, tag="acc1")
                    isl = slice(chunk * 512, (chunk + 1) * 512)
                    for c in range(NBLK):
                        nc.tensor.matmul(
                            acc1,
                            lhsT=s2b[:, c * 128:(c + 1) * 128],
                            rhs=ptp[:, c, isl],
                            start=(c == 0), stop=(c == NBLK - 1),
                        )
                    epilogue2(chunk, acc1)

    nc.compile()
    return nc


_nc_cache = None


def _run(seq_1, seq_2, trace=False):
    global _nc_cache
    if _nc_cache is None:
        _nc_cache = _build()
    nc = _nc_cache
    seq_1 = np.ascontiguousarray(np.asarray(seq_1, dtype=np.float32))
    seq_2 = np.ascontiguousarray(np.asarray(seq_2, dtype=np.float32))
    in_maps = [{"seq_1": seq_1[b], "seq_2": seq_2[b]} for b in range(B)]
    res = run_bass_kernel_spmd(nc, in_maps, core_ids=list(range(B)), trace=trace)
    out1 = np.stack([res.results[b]["out1"] for b in range(B)])
    out2 = np.stack([res.results[b]["out2"] for b in range(B)])
    return (out1, out2), res


def kernel(seq_1, seq_2):
    return _run(seq_1, seq_2)[0]
